# revision 6
# baseline (speedup 1.0000x reference)
"""Contrastive flow loss on 8 Trainium2 NeuronCores.

Math (faithful to the reference):
    z_norm = z / max(||z||, eps)
    sim    = z_norm @ z_norm.T / T            (B x B, symmetric)
    pos_mask[i,j] = (a_i . a_j == 4) & (i != j)
                  = p_i * p_j off-diagonal, with p_i = all-ones(attr row i)
                    (exact: binary attrs, dot of 4 0/1 terms == 4 iff both rows
                     are all ones)
    all_sum_i = sum_j exp(sim_ij) - exp(sim_ii)
    pos_sum_i = p_i ? sum_{j!=i, p_j=1} exp(sim_ij) + (B - P + 1) : B
    loss_i    = log(all_sum_i) - log(max(pos_sum_i, eps))
    loss      = mean over valid rows (num_pos > 0)

Device strategy (data-parallel over row blocks, 1024 rows/core):
  Each core receives z rolled so that ITS row block sits at rows 0..1023
  (identical SPMD program, no per-core constants).  It normalizes the full
  z, transposes it to zT [128, 8192] (feature-major), then for its block:
      sim block  = 128x512 f32r matmuls (PE)
      diag killed by adding -1e30 to sim (exp -> 0)       (DVE)
      E = exp(sim/T) as bf16                               (ACT)
      column sums [1s | p]^T @ E accumulated in PSUM       (PE)
  Because diag-zeroed E is exactly symmetric, the summed COLUMN sums over
  all cores equal the ROW sums the reference needs:
      u_j = sum_i E'_ij = all_sum_j,  s_j = sum_i p_i E'_ij = masked row sum.
  The host just adds up the 8 partial [2, 8192] outputs (the "all-reduce")
  and finishes the O(B) scalar arithmetic.
"""

import numpy as np

B = 8192          # batch rows
D = 128           # feature dim
A = 4             # attribute dim
NCORES = 8
RB = B // NCORES  # rows per core
NRT = RB // 128   # 128-row tiles per core block
CW = 1024         # column group width (one ACT op)
NG = B // CW      # column groups
TEMP = 0.07
EPS = 1e-12

_CACHE = {}


def _build():
    from contextlib import ExitStack  # noqa: F401

    import concourse.bacc as bacc
    import concourse.tile as tile
    from concourse import mybir
    from concourse.masks import make_identity

    f32 = mybir.dt.float32
    bf16 = mybir.dt.bfloat16
    f32r = mybir.dt.float32r
    Alu = mybir.AluOpType
    Act = mybir.ActivationFunctionType

    nc = bacc.Bacc("TRN2", debug=False)
    z_in = nc.dram_tensor("z_full", [B, D], f32, kind="ExternalInput").ap()
    a_in = nc.dram_tensor("attr_blk", [RB, A], f32, kind="ExternalInput").ap()
    cs_out = nc.dram_tensor("csum", [2, B], f32, kind="ExternalOutput").ap()

    with tile.TileContext(nc) as tc:
        with (
            tc.tile_pool(name="const", bufs=1) as const,
            tc.tile_pool(name="zTbuf", bufs=1) as zTp,
        ):
            # --- constants ---
            ident = const.tile([128, 128], f32)
            make_identity(nc, ident)
            negI = const.tile([128, 128], f32)
            nc.gpsimd.memset(negI, 0.0)
            nc.gpsimd.affine_select(
                out=negI,
                in_=negI,
                compare_op=Alu.not_equal,
                fill=-1e30,
                base=0,
                pattern=[[-1, 128]],
                channel_multiplier=1,
            )

            # --- p for this core's rows -> colsum stationaries W_r = [1 | p] ---
            attr_t = const.tile([128, NRT, A], f32)
            nc.sync.dma_start(out=attr_t, in_=a_in.rearrange("(r p) a -> p r a", p=128))
            asum = const.tile([128, NRT], f32)
            nc.vector.tensor_reduce(
                out=asum, in_=attr_t, axis=mybir.AxisListType.X, op=Alu.add
            )
            pvec = const.tile([128, NRT], f32)
            # attr sums are exact small ints; relu(sum - 3) == 1 iff sum == 4
            bias_m3 = const.tile([128, 1], f32)
            nc.vector.memset(bias_m3, -3.0)
            nc.scalar.activation(
                out=pvec, in_=asum, func=Act.Relu, bias=bias_m3, scale=1.0
            )
            W = const.tile([128, NRT, 2], bf16)
            nc.vector.memset(W, 1.0)
            for r in range(NRT):
                nc.vector.tensor_copy(out=W[:, r, 1:2], in_=pvec[:, r : r + 1])

            # --- phase A: normalize z and build zT [128 feat, B rows] ---
            zT = zTp.tile([128, B], f32r)
            with (
                tc.tile_pool(name="znat", bufs=1) as znatp,
                tc.tile_pool(name="sqs", bufs=2) as sqp,
                tc.tile_pool(name="zns", bufs=3) as znp,
                tc.tile_pool(name="normv", bufs=1) as normp,
                tc.tile_pool(name="tps", bufs=2, space="PSUM") as tpp,
            ):
                znat = znatp.tile([128, B // 128, 128], f32)
                zre = z_in.rearrange("(n p) d -> p n d", p=128)
                for g in range(8):
                    nc.sync.dma_start(
                        out=znat[:, g * 8 : (g + 1) * 8, :],
                        in_=zre[:, g * 8 : (g + 1) * 8, :],
                    )
                sumsq = normp.tile([128, B // 128], f32)
                for g in range(8):
                    sq = sqp.tile([128, 8, 128], f32)
                    nc.vector.tensor_mul(
                        out=sq,
                        in0=znat[:, g * 8 : (g + 1) * 8, :],
                        in1=znat[:, g * 8 : (g + 1) * 8, :],
                    )
                    nc.vector.tensor_reduce(
                        out=sumsq[:, g * 8 : (g + 1) * 8],
                        in_=sq,
                        axis=mybir.AxisListType.X,
                        op=Alu.add,
                    )
                rno = normp.tile([128, B // 128], f32)
                nc.scalar.activation(out=rno, in_=sumsq, func=Act.Sqrt)
                nc.vector.tensor_scalar_max(out=rno, in0=rno, scalar1=float(EPS))
                rre = normp.tile([128, B // 128], f32)
                nc.vector.reciprocal(out=rre, in_=rno)
                for n in range(B // 128):
                    zn = znp.tile([128, 128], f32)
                    nc.vector.tensor_scalar_mul(
                        out=zn, in0=znat[:, n, :], scalar1=rre[:, n : n + 1]
                    )
                    pt = tpp.tile([128, 128], f32)
                    nc.tensor.transpose(pt, zn, ident)
                    nc.vector.tensor_copy(out=zT[:, n * 128 : (n + 1) * 128], in_=pt)

            # --- phase B: sim block, exp, column sums ---
            with (
                tc.tile_pool(name="simps", bufs=2, space="PSUM") as simp,
                tc.tile_pool(name="csps", bufs=2, space="PSUM") as csp,
                tc.tile_pool(name="esb", bufs=3) as ep,
                tc.tile_pool(name="cso", bufs=1) as csop,
            ):
                csum_sb = csop.tile([2, B], f32)
                for g in range(NG):
                    cs = csp.tile([2, CW], f32)
                    for r in range(NRT):
                        sim = simp.tile([128, CW], f32)
                        for h in range(CW // 512):
                            n = g * (CW // 512) + h
                            nc.tensor.matmul(
                                sim[:, h * 512 : (h + 1) * 512],
                                lhsT=zT[:, r * 128 : (r + 1) * 128],
                                rhs=zT[:, n * 512 : (n + 1) * 512],
                                start=True,
                                stop=True,
                            )
                        if (r * 128) // CW == g:
                            off = (r * 128) % CW
                            nc.vector.tensor_add(
                                out=sim[:, off : off + 128],
                                in0=sim[:, off : off + 128],
                                in1=negI,
                            )
                        E = ep.tile([128, CW], bf16)
                        nc.scalar.activation(
                            out=E, in_=sim, func=Act.Exp, scale=float(1.0 / TEMP)
                        )
                        for h in range(CW // 512):
                            nc.tensor.matmul(
                                cs[:, h * 512 : (h + 1) * 512],
                                lhsT=W[:, r, :],
                                rhs=E[:, h * 512 : (h + 1) * 512],
                                start=(r == 0),
                                stop=(r == NRT - 1),
                            )
                    nc.vector.tensor_copy(
                        out=csum_sb[:, g * CW : (g + 1) * CW], in_=cs
                    )
                nc.sync.dma_start(out=cs_out, in_=csum_sb)

    nc.compile()
    return nc


def _get_nc():
    if "nc" not in _CACHE:
        _CACHE["nc"] = _build()
    return _CACHE["nc"]


def kernel(z_flowed: np.ndarray, attributes: np.ndarray) -> np.ndarray:
    from concourse.bass_utils import run_bass_kernel_spmd

    z = np.ascontiguousarray(np.asarray(z_flowed, dtype=np.float32))
    attrs = np.ascontiguousarray(np.asarray(attributes, dtype=np.float32))

    nc = _get_nc()
    in_maps = []
    for c in range(NCORES):
        in_maps.append(
            {
                "z_full": np.roll(z, -c * RB, axis=0),
                "attr_blk": np.ascontiguousarray(attrs[c * RB : (c + 1) * RB]),
            }
        )
    res = run_bass_kernel_spmd(nc, in_maps, list(range(NCORES)))
    _CACHE["last_result"] = res

    u = np.zeros(B, np.float64)
    s = np.zeros(B, np.float64)
    for c in range(NCORES):
        cs = res.results[c]["csum"]
        u += np.roll(cs[0].astype(np.float64), c * RB)
        s += np.roll(cs[1].astype(np.float64), c * RB)

    # host-side gather / final O(B) scalar math (the "all-reduce" step)
    p = attrs.sum(axis=1) == float(A)
    P = int(p.sum())
    all_sum = u
    pos_sum = np.where(p, s + float(B - P + 1), float(B))
    num_pos = np.where(p, P - 1, 0)
    valid = (num_pos > 0) & (all_sum > 0) & (pos_sum > 0)
    with np.errstate(divide="ignore", invalid="ignore"):
        loss_i = np.log(all_sum) - np.log(np.maximum(pos_sum, EPS))
    cnt = int(valid.sum())
    total = float(np.where(valid, loss_i, 0.0).sum())
    loss = total / max(cnt, 1) if cnt > 0 else 0.0
    return np.asarray(loss, dtype=np.float32)


# revision 10
# speedup vs baseline: 60.4012x; 60.4012x over previous
"""Contrastive flow loss on 8 Trainium2 NeuronCores.

Math (faithful to the reference):
    z_norm = z / max(||z||, eps)
    sim    = z_norm @ z_norm.T / T            (B x B, symmetric)
    pos_mask[i,j] = (a_i . a_j == 4) & (i != j)
                  = p_i * p_j off-diagonal, with p_i = all-ones(attr row i)
                    (exact: binary attrs, dot of 4 0/1 terms == 4 iff both rows
                     are all ones)
    all_sum_i = sum_j exp(sim_ij) - exp(sim_ii)
    pos_sum_i = p_i ? sum_{j!=i, p_j=1} exp(sim_ij) + (B - P + 1) : B
    loss_i    = log(all_sum_i) - log(max(pos_sum_i, eps))
    loss      = mean over valid rows (num_pos > 0)

Device strategy (data-parallel over row blocks, 1024 rows/core):
  Each core receives z rolled so that ITS row block sits at rows 0..1023
  (identical SPMD program, no per-core constants).  Work is emitted in 8
  column groups of 1024 so normalization/transposition of group g+1 overlaps
  the matmul/exp pipeline of group g:
      sim block  = 128x512 f32r matmuls (PE)
      diag killed by adding -1e30 to sim (exp -> 0); in the rolled layout the
        whole block diagonal lives in column group 0                 (DVE)
      E = exp(sim/T) as bf16                                         (ACT)
      column sums [1s | p]^T @ E accumulated in PSUM                 (PE)
  rsqrt for normalization is computed as exp(-0.5*ln(max(sumsq,1e-24))) so
  every ACT op lives in the single `natural_log_exp_and_others` table set
  (no ~2.7us table reloads when phases interleave).  The clamp matches the
  reference: for sumsq < eps^2 it yields exactly 1/eps.
  Because diag-zeroed E is exactly symmetric, the summed COLUMN sums over
  all cores equal the ROW sums the reference needs:
      u_j = sum_i E'_ij = all_sum_j,  s_j = sum_i p_i E'_ij = masked row sum.
  The host adds the 8 partial [2, 8192] outputs (the "all-reduce") and
  finishes the O(B) scalar arithmetic.
"""

import numpy as np

B = 8192          # batch rows
D = 128           # feature dim
A = 4             # attribute dim
NCORES = 8
RB = B // NCORES  # rows per core
NRT = RB // 128   # 128-row tiles per core block
CW = 1024         # column group width (one ACT op)
NG = B // CW      # column groups
TEMP = 0.07
EPS = 1e-12

_CACHE = {}


def _build(repeat: int = 1):
    import concourse.bacc as bacc
    import concourse.tile as tile
    from concourse import mybir
    from concourse.masks import make_identity

    f32 = mybir.dt.float32
    bf16 = mybir.dt.bfloat16
    f32r = mybir.dt.float32r
    Alu = mybir.AluOpType
    Act = mybir.ActivationFunctionType

    nc = bacc.Bacc("TRN2", debug=False)
    z_in = nc.dram_tensor("z_full", [B, D], f32, kind="ExternalInput").ap()
    a_in = nc.dram_tensor("attr_blk", [RB, A], f32, kind="ExternalInput").ap()
    cs_out = nc.dram_tensor("csum", [2, B], f32, kind="ExternalOutput").ap()

    with tile.TileContext(nc) as tc:
        with (
            tc.tile_pool(name="const", bufs=1) as const,
            tc.tile_pool(name="zTbuf", bufs=1) as zTp,
            tc.tile_pool(name="znatp", bufs=2) as znatp,
            tc.tile_pool(name="sqp", bufs=2) as sqp,
            tc.tile_pool(name="normp", bufs=2) as normp,
            tc.tile_pool(name="znp", bufs=3) as znp,
            tc.tile_pool(name="ps", bufs=2, space="PSUM") as psp,
            tc.tile_pool(name="csps", bufs=1, space="PSUM") as csp,
            tc.tile_pool(name="esb", bufs=3) as ep,
            tc.tile_pool(name="cso", bufs=1) as csop,
        ):
            # --- constants ---
            ident = const.tile([128, 128], f32)
            make_identity(nc, ident)
            negI = const.tile([128, 128], f32)
            nc.gpsimd.memset(negI, 0.0)
            nc.gpsimd.affine_select(
                out=negI,
                in_=negI,
                compare_op=Alu.not_equal,
                fill=-1e30,
                base=0,
                pattern=[[-1, 128]],
                channel_multiplier=1,
            )

            # --- p for this core's rows -> colsum stationaries W_r = [1 | p] ---
            attr_t = const.tile([128, NRT, A], f32)
            nc.sync.dma_start(out=attr_t, in_=a_in.rearrange("(r p) a -> p r a", p=128))
            asum = const.tile([128, NRT], f32)
            nc.vector.tensor_reduce(
                out=asum, in_=attr_t, axis=mybir.AxisListType.X, op=Alu.add
            )
            pvec = const.tile([128, NRT], f32)
            # attr sums are exact small ints; relu(sum - 3) == 1 iff sum == 4
            bias_m3 = const.tile([128, 1], f32)
            nc.vector.memset(bias_m3, -3.0)
            nc.scalar.activation(
                out=pvec, in_=asum, func=Act.Relu, bias=bias_m3, scale=1.0
            )
            W = const.tile([128, NRT, 2], bf16)
            nc.vector.memset(W, 1.0)
            for r in range(NRT):
                nc.vector.tensor_copy(out=W[:, r, 1:2], in_=pvec[:, r : r + 1])

            zre = z_in.rearrange("(n p) d -> p n d", p=128)

            def body():
                zTt = []
                csum_sb = csop.tile([2, B], f32, tag="csum_sb")
                for gt in range(NG):
                    # --- phase A for column group gt: normalize + transpose ---
                    znc = znatp.tile([128, 8, 128], f32, tag="znat")
                    nc.sync.dma_start(out=znc, in_=zre[:, gt * 8 : (gt + 1) * 8, :])
                    sq = sqp.tile([128, 8, 128], f32, tag="sq")
                    nc.vector.tensor_mul(out=sq, in0=znc, in1=znc)
                    ss = normp.tile([128, 8], f32, tag="ss")
                    nc.vector.tensor_reduce(
                        out=ss, in_=sq, axis=mybir.AxisListType.X, op=Alu.add
                    )
                    nc.vector.tensor_scalar_max(out=ss, in0=ss, scalar1=1e-24)
                    lnv = normp.tile([128, 8], f32, tag="lnv")
                    nc.scalar.activation(out=lnv, in_=ss, func=Act.Ln)
                    rn = normp.tile([128, 8], f32, tag="rn")
                    # rn = exp(-0.5*ln(ss)) = 1/sqrt(ss); table set shared w/ Exp
                    nc.scalar.activation(out=rn, in_=lnv, func=Act.Exp, scale=-0.5)
                    zTg = zTp.tile([128, CW], f32r, tag=f"zt{gt}")
                    for j in range(8):
                        zn = znp.tile([128, 128], f32, tag="zn")
                        nc.vector.tensor_scalar_mul(
                            out=zn, in0=znc[:, j, :], scalar1=rn[:, j : j + 1]
                        )
                        pt = psp.tile([128, 128], f32, tag="tp")
                        nc.tensor.transpose(pt, zn, ident)
                        nc.vector.tensor_copy(
                            out=zTg[:, j * 128 : (j + 1) * 128], in_=pt
                        )
                    zTt.append(zTg)

                    # --- phase B for column group gt ---
                    cs = csp.tile([2, CW], f32, tag="cs")
                    for r in range(NRT):
                        sim = psp.tile([128, CW], f32, tag="sim")
                        for h in range(CW // 512):
                            nc.tensor.matmul(
                                sim[:, h * 512 : (h + 1) * 512],
                                lhsT=zTt[0][:, r * 128 : (r + 1) * 128],
                                rhs=zTg[:, h * 512 : (h + 1) * 512],
                                start=True,
                                stop=True,
                            )
                        if gt == 0:
                            # block diagonal: local rows r*128.. vs same cols
                            off = r * 128
                            nc.vector.tensor_add(
                                out=sim[:, off : off + 128],
                                in0=sim[:, off : off + 128],
                                in1=negI,
                            )
                        E = ep.tile([128, CW], bf16, tag="E")
                        nc.scalar.activation(
                            out=E, in_=sim, func=Act.Exp, scale=float(1.0 / TEMP)
                        )
                        for h in range(CW // 512):
                            nc.tensor.matmul(
                                cs[:, h * 512 : (h + 1) * 512],
                                lhsT=W[:, r, :],
                                rhs=E[:, h * 512 : (h + 1) * 512],
                                start=(r == 0),
                                stop=(r == NRT - 1),
                            )
                    nc.vector.tensor_copy(
                        out=csum_sb[:, gt * CW : (gt + 1) * CW], in_=cs
                    )
                nc.sync.dma_start(out=cs_out, in_=csum_sb)

            for _rep in range(repeat):
                body()

    nc.compile()
    return nc


def _get_nc(repeat: int = 1):
    key = ("nc", repeat)
    if key not in _CACHE:
        _CACHE[key] = _build(repeat)
    return _CACHE[key]


def kernel(z_flowed: np.ndarray, attributes: np.ndarray) -> np.ndarray:
    from concourse.bass_utils import run_bass_kernel_spmd

    z = np.ascontiguousarray(np.asarray(z_flowed, dtype=np.float32))
    attrs = np.ascontiguousarray(np.asarray(attributes, dtype=np.float32))

    nc = _get_nc()
    in_maps = []
    for c in range(NCORES):
        in_maps.append(
            {
                "z_full": np.roll(z, -c * RB, axis=0),
                "attr_blk": np.ascontiguousarray(attrs[c * RB : (c + 1) * RB]),
            }
        )
    res = run_bass_kernel_spmd(nc, in_maps, list(range(NCORES)))
    _CACHE["last_result"] = res

    u = np.zeros(B, np.float64)
    s = np.zeros(B, np.float64)
    for c in range(NCORES):
        cs = res.results[c]["csum"]
        u += np.roll(cs[0].astype(np.float64), c * RB)
        s += np.roll(cs[1].astype(np.float64), c * RB)

    # host-side gather / final O(B) scalar math (the "all-reduce" step)
    p = attrs.sum(axis=1) == float(A)
    P = int(p.sum())
    all_sum = u
    pos_sum = np.where(p, s + float(B - P + 1), float(B))
    num_pos = np.where(p, P - 1, 0)
    valid = (num_pos > 0) & (all_sum > 0) & (pos_sum > 0)
    with np.errstate(divide="ignore", invalid="ignore"):
        loss_i = np.log(all_sum) - np.log(np.maximum(pos_sum, EPS))
    cnt = int(valid.sum())
    total = float(np.where(valid, loss_i, 0.0).sum())
    loss = total / max(cnt, 1) if cnt > 0 else 0.0
    return np.asarray(loss, dtype=np.float32)
